# revision 10
# baseline (speedup 1.0000x reference)
"""Trainium2 Bass kernel for nn_AutoregressiveFlowLayer (v22).

Computes, for batch x [B, D] and R ragged regions (padded to RMAX):
    xg   = x[:, idx] * valid                       [B, R, RMAX]
    h1   = relu(xg @ (W1*M1))                      [B, R, 128]
    h2   = relu(h1 @ (W2*M2))                      [B, R, 128]
    out  = h2 @ (Wout*Mout) -> (shift, log_s)      [B, R, RMAX, 2]
    u    = (xg - shift) * exp(-log_s)
    ll   = sum(valid * (-0.5 u^2 - 0.5 log(2pi) - log_s), -1)   [B, R, 1]

Sharding: data-parallel over batch across 8 NeuronCores; weights replicated.
idx/valid are baked into the compiled program (recompiled if they change).

v22 history:
  v20 (102.6us) was elementwise-bound: ACT 66.6us + DVE 66.1us of ~690ns
  PSUM-evacuation ops; PE 42us HAM-throttled from the resulting stalls.
  v21 (122.6us) halved the evacuation op count with 2-bank pair tiles but
  its 2-pair php ring serialized PE<->evac into lock-step (~43% engine
  utilization).  v22 keeps the pair savings and restores decoupling:
  - php = 3 pair bufs [128,1024]; per step 5 pair allocs: L1A, L1B,
    L2A, L2B, and L3 (logs half | shift half) - shift/logs fold into
    the same ring instead of owning dedicated banks.
  - p = q + logs is gone; instead logs is evacuated to SBUF (single-src
    op, schedulable on either engine) and the reduce becomes TWO
    accumulating matmuls ll = -(v.q) - (v.logs).  This frees the L3
    pair within its own step, breaking the cross-step dependency cycle
    through the q chain.
  - reduce matmuls of 4 consecutive steps write one pll bank at
    partition offsets 0/32/64/96 (tile_position=(0,32j), M=32 with
    zero-padded negv columns so the bank is fully initialized); one
    Identity+bias copy-out on ACT + 4 small DMAs per 4 steps.
  - PSUM banks: 3 pairs (6) + pll 2 = 8.
"""

import sys

import numpy as np

_TRN_REPO = "/opt/trn_rl_repo"
if _TRN_REPO not in sys.path:
    sys.path.insert(0, _TRN_REPO)

D = 1024
R = 32
RMAX = 32
H1 = 128
H2 = 128
B = 8192
NCORES = 8
BC = B // NCORES          # batch per core
NG = R // 4               # 8 groups of 4 regions
BH = 512                  # batch half-tile (one PSUM bank of fp32)
LN2PI = float(np.log(2.0 * np.pi))
EXP_BIAS = float(-0.5 * np.log(2.0))  # exp(-logs + b) = exp(-logs)/sqrt(2)

_cache = {}


def _build_program(idx, valid):
    import concourse.mybir as mybir
    import concourse.tile as tile
    from concourse import bacc

    dt = mybir.dt
    AF = mybir.ActivationFunctionType

    nc = bacc.Bacc("TRN2", target_bir_lowering=False, debug=False)

    # ---- DRAM tensors (per-core inputs) ----
    xg_d = nc.dram_tensor("xg", [128, NG * BC], dt.bfloat16, kind="ExternalInput").ap()
    w1 = nc.dram_tensor("w1", [128, NG, 128], dt.bfloat16, kind="ExternalInput").ap()
    w2 = nc.dram_tensor("w2", [128, R, 128], dt.bfloat16, kind="ExternalInput").ap()
    w3 = nc.dram_tensor("w3", [128, R, 64], dt.bfloat16, kind="ExternalInput").ap()
    negv = nc.dram_tensor("negv", [128, NG, 32], dt.bfloat16, kind="ExternalInput").ap()
    cb = nc.dram_tensor("cb", [128, 4], dt.float32, kind="ExternalInput").ap()
    out_d = nc.dram_tensor("out", [4, NG * BC], dt.float32, kind="ExternalOutput").ap()

    from contextlib import ExitStack

    with tile.TileContext(nc) as tc, ExitStack() as ctx:
        singles = ctx.enter_context(tc.tile_pool(name="singles", bufs=1))
        hs = ctx.enter_context(tc.tile_pool(name="hs", bufs=7))
        es = ctx.enter_context(tc.tile_pool(name="es", bufs=10))
        # PSUM: php = 3 pair slabs [128,1024] (2 banks each) cycling
        # L1A,L1B,L2A,L2B,L3 each step; pll = 2 banks, each collecting 4
        # steps' [4,512] ll rows at partition offsets 0/32/64/96.
        php = ctx.enter_context(tc.tile_pool(name="php", bufs=3, space="PSUM"))
        pll = ctx.enter_context(tc.tile_pool(name="pll", bufs=2, space="PSUM"))

        # ---- load constants into SBUF ----
        w1s = singles.tile([128, NG, 128], dt.bfloat16)
        w2s = singles.tile([128, R, 128], dt.bfloat16)
        w3s = singles.tile([128, R, 64], dt.bfloat16)
        negvs = singles.tile([128, NG, 32], dt.bfloat16)
        cbs = singles.tile([128, 4], dt.float32)

        # gathered ragged inputs (bf16, host-side gather): one tile per
        # group so compute on group g only waits for its own slab.
        xgb = []
        for g in range(NG):
            t = singles.tile([128, 1, BC], dt.bfloat16, tag=f"xgb{g}")
            xgb.append(t)

        # startup-critical slices first: step (0,0) needs only the first
        # batch half of group 0 and group 0's weights (~300KB), not the
        # full 3.75MB input set -> the first matmul starts ~3us earlier.
        nc.sync.dma_start(out=xgb[0][:, :, 0:BH], in_=xg_d[:, 0:BH])
        nc.sync.dma_start(out=w1s[:, 0, :], in_=w1[:, 0, :])
        nc.sync.dma_start(out=w2s[:, 0:4, :], in_=w2[:, 0:4, :])
        nc.sync.dma_start(out=w3s[:, 0:4, :], in_=w3[:, 0:4, :])
        nc.sync.dma_start(out=xgb[0][:, :, BH:BC], in_=xg_d[:, BH:BC])
        nc.sync.dma_start(out=negvs[:], in_=negv)
        nc.sync.dma_start(out=cbs[:], in_=cb)
        nc.sync.dma_start(out=xgb[1][:], in_=xg_d[:, BC:2 * BC])
        nc.sync.dma_start(out=w1s[:, 1:NG, :], in_=w1[:, 1:NG, :])
        nc.sync.dma_start(out=w2s[:, 4:R, :], in_=w2[:, 4:R, :])
        nc.sync.dma_start(out=w3s[:, 4:R, :], in_=w3[:, 4:R, :])
        for g in range(2, NG):
            nc.sync.dma_start(out=xgb[g][:], in_=xg_d[:, g * BC:(g + 1) * BC])

        # per-partition constant bias for the exp
        ebias = singles.tile([128, 1], dt.float32)
        nc.vector.memset(ebias[:], EXP_BIAS)

        # warm-load dummies: pull ACT_TABLE_LOAD + Q7 ucode load into the
        # preamble dead time.
        wl0 = singles.tile([1, 1], dt.bfloat16)
        nc.scalar.activation(wl0[:], ebias[0:1, 0:1], AF.Exp)
        wl1 = singles.tile([1, 1], dt.bfloat16)
        nc.gpsimd.tensor_mul(wl1[:], ebias[0:1, 0:1], ebias[0:1, 0:1])

        nh = BC // BH  # halves per core
        nsteps = NG * nh

        def relu(on_act, dst, src):
            if on_act:
                nc.scalar.activation(dst, src, AF.Relu)
            else:
                nc.vector.tensor_scalar_max(dst, src, 0.0)

        # deferred reduce of step `prev`: TWO accumulating matmuls
        # ll4 = -(v.q) - (v.logs) into the shared pll bank at partition
        # offset 32*(s%4) (M=32, cols 4..31 of negv are zero so the
        # whole bank stays initialized).  Every 4 steps: one ACT
        # Identity+bias copy-out + 4 small DMAs.
        state = {"ll": None}

        def emit_reduce(prev):
            qt, lgev, s = prev
            g = s // nh
            j = s % 4
            if j == 0:
                state["ll"] = pll.tile([128, BH], dt.float32, tag="ll",
                                       name="llt")
            llp = state["ll"][32 * j:32 * (j + 1), 0:BH]
            nc.tensor.matmul(
                out=llp, lhsT=negvs[:, g, :], rhs=qt[:],
                start=True, stop=False, tile_position=(0, 32 * j),
            )
            nc.tensor.matmul(
                out=llp, lhsT=negvs[:, g, :], rhs=lgev[:],
                start=False, stop=True, tile_position=(0, 32 * j),
            )
            if j == 3:
                c = s // 4
                lls = singles.tile([128, BH], dt.float32, tag=f"lls{c}",
                                   name="lls")
                nc.scalar.activation(lls[:], state["ll"][:], AF.Identity,
                                     bias=cbs[:, c:c + 1])
                for jj in range(4):
                    nc.sync.dma_start(
                        out=out_d[:, 2 * c * BC + jj * BH:
                                  2 * c * BC + (jj + 1) * BH],
                        in_=lls[32 * jj:32 * jj + 4, :])

        prev = None
        for step in range(nsteps):
            g, h = step // nh, step % nh
            b0 = h * BH
            xgbs = xgb[g][:, 0, b0:b0 + BH]

            # engine split: True = ACT.  DVE carries sub (+ lgev on odd
            # steps), ACT carries exp (+ lgev on even steps, copy-out
            # every 4th).  Pair relus split 2/2.
            RELU_ACT = (True, False, True, False)
            LGEV_ACT = (step % 2 == 0)

            # ---- L1: two pair slabs, 4 row-tiled K=32 matmuls
            l1p = [php.tile([128, 2 * BH], dt.float32, tag="ph", name="l1p")
                   for _ in range(2)]
            for j in range(4):
                nc.tensor.matmul(
                    out=l1p[j // 2][:, BH * (j % 2):BH * (j % 2 + 1)],
                    lhsT=w1s[32 * j:32 * (j + 1), g, :],
                    rhs=xgbs[32 * j:32 * (j + 1), :],
                    start=True, stop=True,
                    tile_position=(32 * j, 0),
                )
            h1sb = []
            for p in range(2):
                ht = hs.tile([128, 2 * BH], dt.bfloat16, tag="hsb",
                             name="h1t")
                relu(RELU_ACT[p], ht[:], l1p[p][:])
                h1sb.append(ht)

            # ---- L2: two pair slabs, 4 dense K=128 matmuls
            l2p = [php.tile([128, 2 * BH], dt.float32, tag="ph", name="l2p")
                   for _ in range(2)]
            for j in range(4):
                nc.tensor.matmul(
                    out=l2p[j // 2][:, BH * (j % 2):BH * (j % 2 + 1)],
                    lhsT=w2s[:, 4 * g + j, :],
                    rhs=h1sb[j // 2][:, BH * (j % 2):BH * (j % 2 + 1)],
                    start=True, stop=True,
                    tile_position=(0, 0),
                )
            h2sb = []
            for p in range(2):
                ht = hs.tile([128, 2 * BH], dt.bfloat16, tag="hsb",
                             name="h2t")
                relu(RELU_ACT[2 + p], ht[:], l2p[p][:])
                h2sb.append(ht)

            # ---- L3: ONE pair slab [logs | shift], col-tiled M=32
            # matmuls.  Logs half first so ACT's exp starts earlier.
            l3p = php.tile([128, 2 * BH], dt.float32, tag="ph", name="l3p")
            lgsl = l3p[:, 0:BH]
            shsl = l3p[:, BH:2 * BH]
            for j in range(4):
                nc.tensor.matmul(
                    out=lgsl[32 * j:32 * (j + 1), :],
                    lhsT=w3s[:, 4 * g + j, 32:64],
                    rhs=h2sb[j // 2][:, BH * (j % 2):BH * (j % 2 + 1)],
                    start=True, stop=True,
                    tile_position=(0, 32 * j),
                )
            for j in range(4):
                nc.tensor.matmul(
                    out=shsl[32 * j:32 * (j + 1), :],
                    lhsT=w3s[:, 4 * g + j, 0:32],
                    rhs=h2sb[j // 2][:, BH * (j % 2):BH * (j % 2 + 1)],
                    start=True, stop=True,
                    tile_position=(0, 32 * j),
                )

            # reduce of the PREVIOUS step: q/logs_sb are ready by now,
            # and pll (bufs=2) never gates the php ring.
            if prev is not None:
                emit_reduce(prev)

            # E' = exp(-logs)/sqrt(2)  (ACT)
            et = es.tile([128, BH], dt.bfloat16, tag="et")
            nc.scalar.activation(et[:], lgsl, AF.Exp,
                                 bias=ebias[:], scale=-1.0)
            # logs -> SBUF bf16 for the reduce (frees the L3 pair this
            # step; engine alternates for balance)
            lgev = es.tile([128, BH], dt.bfloat16, tag="lgev")
            if LGEV_ACT:
                nc.scalar.activation(lgev[:], lgsl, AF.Identity)
            else:
                nc.vector.tensor_copy(lgev[:], lgsl)
            # d = xg - shift  (DVE, PSUM operand)
            dtl = es.tile([128, BH], dt.bfloat16, tag="dt")
            nc.vector.tensor_sub(dtl[:], xgbs, shsl)
            # u' = d * E'   ;  q = u'^2 = 0.5 u^2   (GPSIMD, SBUF-only)
            ut = es.tile([128, BH], dt.bfloat16, tag="ut")
            nc.gpsimd.tensor_mul(ut[:], dtl[:], et[:])
            qt = es.tile([128, BH], dt.bfloat16, tag="qt")
            nc.gpsimd.tensor_mul(qt[:], ut[:], ut[:])

            prev = (qt, lgev, step)

        emit_reduce(prev)

    nc.compile()
    return nc


def _host_prep(inputs, W1, W2, Wout, idx, valid, M1, M2, Mout):
    import ml_dtypes

    bf16 = ml_dtypes.bfloat16
    f32 = np.float32

    idx = np.asarray(idx)
    valid = np.asarray(valid)
    vf = valid.astype(f32)                                  # [R, RMAX]
    Wm1 = (np.asarray(W1) * np.asarray(M1)).astype(f32)     # [R, 32, 128]
    Wm2 = (np.asarray(W2) * np.asarray(M2)).astype(f32)     # [R, 128, 128]
    Wm3 = (np.asarray(Wout) * np.asarray(Mout)).astype(f32)  # [R, 128, 64]
    Wsh = Wm3[:, :, 0::2]                                   # [R, 128, 32]
    Wlg = Wm3[:, :, 1::2]                                   # [R, 128, 32]

    w1 = np.zeros((128, NG, 128), f32)
    for g in range(NG):
        for j in range(4):
            w1[32 * j:32 * (j + 1), g, :] = Wm1[4 * g + j]
    w1 = w1.astype(bf16)
    w2 = np.ascontiguousarray(Wm2.transpose(1, 0, 2)).astype(bf16)  # [128,R,128]
    w3 = np.concatenate([Wsh, Wlg], axis=2)                 # [R, 128, 64]
    w3 = np.ascontiguousarray(w3.transpose(1, 0, 2)).astype(bf16)   # [128,R,64]

    negv = np.zeros((128, NG, 32), f32)
    for g in range(NG):
        for j in range(4):
            r = 4 * g + j
            negv[32 * j:32 * (j + 1), g, j] = -vf[r]
    negv = negv.astype(bf16)

    # cb[32*j + i, c] = -0.5*ln(2pi)*sum(v_r) for region r = 4g+i of
    # step s = 4c+j (g = 2c + j//2); the batched ll copy-out adds it as
    # a per-partition scalar.
    cbv = np.zeros((128, 4), f32)
    for c in range(4):
        for j in range(4):
            gg = 2 * c + j // 2
            for i in range(4):
                cbv[32 * j + i, c] = -0.5 * LN2PI * float(vf[4 * gg + i].sum())

    # host-side ragged gather: partition p of group g holds
    # x[:, idx[4g + p//32, p%32]] * valid, transposed to [feat, batch]
    rows = idx.reshape(NG, 4 * RMAX)                        # [NG, 128]
    vflat = vf.reshape(NG, 4 * RMAX)                        # [NG, 128]
    xT = np.asarray(inputs, dtype=f32).T                    # [D, B]
    xg_full = xT[rows.reshape(-1)] * vflat.reshape(-1, 1)   # [NG*128, B]
    xg_full = xg_full.reshape(NG, 128, B).astype(bf16)

    per_core = []
    for c in range(NCORES):
        sl = xg_full[:, :, c * BC:(c + 1) * BC]             # [NG, 128, BC]
        xg = np.ascontiguousarray(sl.transpose(1, 0, 2)).reshape(128, NG * BC)
        per_core.append({
            "xg": xg,
            "w1": w1, "w2": w2, "w3": w3,
            "negv": negv, "cb": cbv,
        })
    return per_core


def _get_compiled(idx, valid):
    key = (np.asarray(idx).tobytes(), np.asarray(valid).tobytes())
    if _cache.get("key") != key:
        _cache["key"] = key
        _cache["nc"] = _build_program(np.asarray(idx), np.asarray(valid))
    return _cache["nc"]


def _assemble(results):
    full = np.zeros((B, R), np.float32)
    for c in range(NCORES):
        o = results[c]["out"]                       # [4, NG*BC]
        o = o.reshape(4, NG, BC).transpose(2, 1, 0).reshape(BC, R)
        full[c * BC:(c + 1) * BC] = o
    return full[..., None]


def kernel(inputs, W1, W2, Wout, idx, valid, M1, M2, Mout):
    from concourse import bass_utils

    nc = _get_compiled(idx, valid)
    in_maps = _host_prep(inputs, W1, W2, Wout, idx, valid, M1, M2, Mout)
    res = bass_utils.run_bass_kernel_spmd(nc, in_maps, core_ids=list(range(NCORES)))
    out = _assemble(res.results)
    _cache["last_exec_time_ns"] = res.exec_time_ns
    return out


def kernel_profiled(inputs, W1, W2, Wout, idx, valid, M1, M2, Mout, tmpdir=None):
    """Like kernel() but requests an NTFF trace; returns (out, exec_time_ns)."""
    from concourse import bass_utils

    nc = _get_compiled(idx, valid)
    in_maps = _host_prep(inputs, W1, W2, Wout, idx, valid, M1, M2, Mout)
    res = bass_utils.run_bass_kernel_spmd(
        nc, in_maps, core_ids=list(range(NCORES)), trace=True, tmpdir=tmpdir,
    )
    out = _assemble(res.results)
    return out, res.exec_time_ns


# revision 14
# speedup vs baseline: 1.1625x; 1.1625x over previous
"""Trainium2 Bass kernel for nn_AutoregressiveFlowLayer (v22).

Computes, for batch x [B, D] and R ragged regions (padded to RMAX):
    xg   = x[:, idx] * valid                       [B, R, RMAX]
    h1   = relu(xg @ (W1*M1))                      [B, R, 128]
    h2   = relu(h1 @ (W2*M2))                      [B, R, 128]
    out  = h2 @ (Wout*Mout) -> (shift, log_s)      [B, R, RMAX, 2]
    u    = (xg - shift) * exp(-log_s)
    ll   = sum(valid * (-0.5 u^2 - 0.5 log(2pi) - log_s), -1)   [B, R, 1]

Sharding: data-parallel over batch across 8 NeuronCores; weights replicated.
idx/valid are baked into the compiled program (recompiled if they change).

v22 history:
  v20 (102.6us) was elementwise-bound: ACT 66.6us + DVE 66.1us of ~690ns
  PSUM-evacuation ops; PE 42us HAM-throttled from the resulting stalls.
  v21 (122.6us) halved the evacuation op count with 2-bank pair tiles but
  its 2-pair php ring serialized PE<->evac into lock-step (~43% engine
  utilization).  v22 keeps the pair savings and restores decoupling:
  - php = 3 pair bufs [128,1024]; per step 5 pair allocs: L1A, L1B,
    L2A, L2B, and L3 (logs half | shift half) - shift/logs fold into
    the same ring instead of owning dedicated banks.
  - p = q + logs is gone; instead logs is evacuated to SBUF (single-src
    op, schedulable on either engine) and the reduce becomes TWO
    accumulating matmuls ll = -(v.q) - (v.logs).  This frees the L3
    pair within its own step, breaking the cross-step dependency cycle
    through the q chain.
  - reduce matmuls of 4 consecutive steps write one pll bank at
    partition offsets 0/32/64/96 (tile_position=(0,32j), M=32 with
    zero-padded negv columns so the bank is fully initialized); one
    Identity+bias copy-out on ACT + 4 small DMAs per 4 steps.
  - PSUM banks: 3 pairs (6) + pll 2 = 8.
"""

import sys

import numpy as np

_TRN_REPO = "/opt/trn_rl_repo"
if _TRN_REPO not in sys.path:
    sys.path.insert(0, _TRN_REPO)

D = 1024
R = 32
RMAX = 32
H1 = 128
H2 = 128
B = 8192
NCORES = 8
BC = B // NCORES          # batch per core
NG = R // 4               # 8 groups of 4 regions
BH = 512                  # batch half-tile (one PSUM bank of fp32)
LN2PI = float(np.log(2.0 * np.pi))
EXP_BIAS = float(-np.log(2.0))  # exp(-2*logs + b) = exp(-2*logs)/2

_cache = {}


def _build_program(idx, valid):
    import concourse.mybir as mybir
    import concourse.tile as tile
    from concourse import bacc

    dt = mybir.dt
    AF = mybir.ActivationFunctionType

    nc = bacc.Bacc("TRN2", target_bir_lowering=False, debug=False)

    # ---- DRAM tensors (per-core inputs) ----
    xg_d = nc.dram_tensor("xg", [128, NG * BC], dt.bfloat16, kind="ExternalInput").ap()
    w1 = nc.dram_tensor("w1", [128, NG, 128], dt.bfloat16, kind="ExternalInput").ap()
    w2 = nc.dram_tensor("w2", [128, R, 128], dt.bfloat16, kind="ExternalInput").ap()
    w3 = nc.dram_tensor("w3", [128, R, 64], dt.bfloat16, kind="ExternalInput").ap()
    negv = nc.dram_tensor("negv", [128, NG, 32], dt.bfloat16, kind="ExternalInput").ap()
    cb = nc.dram_tensor("cb", [128, 4], dt.float32, kind="ExternalInput").ap()
    out_d = nc.dram_tensor("out", [4, NG * BC], dt.float32, kind="ExternalOutput").ap()

    from contextlib import ExitStack

    with tile.TileContext(nc) as tc, ExitStack() as ctx:
        singles = ctx.enter_context(tc.tile_pool(name="singles", bufs=1))
        hs = ctx.enter_context(tc.tile_pool(name="hs", bufs=7))
        es = ctx.enter_context(tc.tile_pool(name="es", bufs=14))
        # PSUM: php = 3 pair slabs [128,1024] (2 banks each) cycling
        # L1A,L1B,L2A,L2B,L3 each step; pll = 2 banks, each collecting 4
        # steps' [4,512] ll rows at partition offsets 0/32/64/96.
        php = ctx.enter_context(tc.tile_pool(name="php", bufs=3, space="PSUM"))
        pll = ctx.enter_context(tc.tile_pool(name="pll", bufs=2, space="PSUM"))

        # ---- load constants into SBUF ----
        w1s = singles.tile([128, NG, 128], dt.bfloat16)
        w2s = singles.tile([128, R, 128], dt.bfloat16)
        w3s = singles.tile([128, R, 64], dt.bfloat16)
        negvs = singles.tile([128, NG, 32], dt.bfloat16)
        cbs = singles.tile([128, 4], dt.float32)

        # gathered ragged inputs (bf16, host-side gather): one tile per
        # group so compute on group g only waits for its own slab.
        xgb = []
        for g in range(NG):
            t = singles.tile([128, 1, BC], dt.bfloat16, tag=f"xgb{g}")
            xgb.append(t)

        # startup-critical slices first: step (0,0) needs only the first
        # batch half of group 0 and group 0's weights (~300KB), not the
        # full 3.75MB input set -> the first matmul starts ~3us earlier.
        nc.sync.dma_start(out=xgb[0][:, :, 0:BH], in_=xg_d[:, 0:BH])
        nc.sync.dma_start(out=w1s[:, 0, :], in_=w1[:, 0, :])
        nc.sync.dma_start(out=w2s[:, 0:4, :], in_=w2[:, 0:4, :])
        nc.sync.dma_start(out=w3s[:, 0:4, :], in_=w3[:, 0:4, :])
        nc.sync.dma_start(out=xgb[0][:, :, BH:BC], in_=xg_d[:, BH:BC])
        nc.sync.dma_start(out=negvs[:], in_=negv)
        nc.sync.dma_start(out=cbs[:], in_=cb)
        nc.sync.dma_start(out=xgb[1][:], in_=xg_d[:, BC:2 * BC])
        nc.sync.dma_start(out=w1s[:, 1:NG, :], in_=w1[:, 1:NG, :])
        nc.sync.dma_start(out=w2s[:, 4:R, :], in_=w2[:, 4:R, :])
        nc.sync.dma_start(out=w3s[:, 4:R, :], in_=w3[:, 4:R, :])
        for g in range(2, NG):
            nc.sync.dma_start(out=xgb[g][:], in_=xg_d[:, g * BC:(g + 1) * BC])

        # per-partition constant bias for the exp
        ebias = singles.tile([128, 1], dt.float32)
        nc.vector.memset(ebias[:], EXP_BIAS)

        # warm-load dummies: pull ACT_TABLE_LOAD + Q7 ucode load into the
        # preamble dead time.
        wl0 = singles.tile([1, 1], dt.bfloat16)
        nc.scalar.activation(wl0[:], ebias[0:1, 0:1], AF.Exp)
        wl1 = singles.tile([1, 1], dt.bfloat16)
        nc.gpsimd.tensor_mul(wl1[:], ebias[0:1, 0:1], ebias[0:1, 0:1])

        nh = BC // BH  # halves per core
        nsteps = NG * nh

        def relu(on_act, dst, src):
            if on_act:
                nc.scalar.activation(dst, src, AF.Relu)
            else:
                nc.vector.tensor_scalar_max(dst, src, 0.0)

        # deferred reduce of step `prev`: TWO accumulating matmuls
        # ll4 = -(v.q) - (v.logs) into the shared pll bank at partition
        # offset 32*(s%4) (M=32, cols 4..31 of negv are zero so the
        # whole bank stays initialized).  Every 4 steps: one ACT
        # Identity+bias copy-out + 4 small DMAs.
        state = {"ll": None}

        def emit_reduce(prev):
            qt, lgev, s = prev
            g = s // nh
            j = s % 4
            if j == 0:
                state["ll"] = pll.tile([128, BH], dt.float32, tag="ll",
                                       name="llt")
            llp = state["ll"][32 * j:32 * (j + 1), 0:BH]
            nc.tensor.matmul(
                out=llp, lhsT=negvs[:, g, :], rhs=qt[:],
                start=True, stop=False, tile_position=(0, 32 * j),
            )
            nc.tensor.matmul(
                out=llp, lhsT=negvs[:, g, :], rhs=lgev[:],
                start=False, stop=True, tile_position=(0, 32 * j),
            )
            if j == 3:
                c = s // 4
                lls = singles.tile([128, BH], dt.float32, tag=f"lls{c}",
                                   name="lls")
                nc.scalar.activation(lls[:], state["ll"][:], AF.Identity,
                                     bias=cbs[:, c:c + 1])
                for jj in range(4):
                    nc.sync.dma_start(
                        out=out_d[:, 2 * c * BC + jj * BH:
                                  2 * c * BC + (jj + 1) * BH],
                        in_=lls[32 * jj:32 * jj + 4, :])

        prevs = []
        for step in range(nsteps):
            g, h = step // nh, step % nh
            b0 = h * BH
            xgbs = xgb[g][:, 0, b0:b0 + BH]

            # engine split: True = ACT.  DVE carries sub (+ lgev on odd
            # steps), ACT carries exp (+ lgev on even steps, copy-out
            # every 4th).  Pair relus split 2/2.
            RELU_ACT = (True, False, True, False)
            LGEV_ACT = (step % 2 == 0)

            # ---- L1: two pair slabs, 4 row-tiled K=32 matmuls
            l1p = [php.tile([128, 2 * BH], dt.float32, tag="ph", name="l1p")
                   for _ in range(2)]
            for j in range(4):
                nc.tensor.matmul(
                    out=l1p[j // 2][:, BH * (j % 2):BH * (j % 2 + 1)],
                    lhsT=w1s[32 * j:32 * (j + 1), g, :],
                    rhs=xgbs[32 * j:32 * (j + 1), :],
                    start=True, stop=True,
                    tile_position=(32 * j, 0),
                )
            h1sb = []
            for p in range(2):
                ht = hs.tile([128, 2 * BH], dt.bfloat16, tag="hsb",
                             name="h1t")
                relu(RELU_ACT[p], ht[:], l1p[p][:])
                h1sb.append(ht)

            # ---- L2: two pair slabs, 4 dense K=128 matmuls
            l2p = [php.tile([128, 2 * BH], dt.float32, tag="ph", name="l2p")
                   for _ in range(2)]
            for j in range(4):
                nc.tensor.matmul(
                    out=l2p[j // 2][:, BH * (j % 2):BH * (j % 2 + 1)],
                    lhsT=w2s[:, 4 * g + j, :],
                    rhs=h1sb[j // 2][:, BH * (j % 2):BH * (j % 2 + 1)],
                    start=True, stop=True,
                    tile_position=(0, 0),
                )
            h2sb = []
            for p in range(2):
                ht = hs.tile([128, 2 * BH], dt.bfloat16, tag="hsb",
                             name="h2t")
                relu(RELU_ACT[2 + p], ht[:], l2p[p][:])
                h2sb.append(ht)

            # ---- L3: ONE pair slab [logs | shift], col-tiled M=32
            # matmuls.  Logs half first so ACT's exp starts earlier.
            l3p = php.tile([128, 2 * BH], dt.float32, tag="ph", name="l3p")
            lgsl = l3p[:, 0:BH]
            shsl = l3p[:, BH:2 * BH]
            for j in range(4):
                nc.tensor.matmul(
                    out=lgsl[32 * j:32 * (j + 1), :],
                    lhsT=w3s[:, 4 * g + j, 32:64],
                    rhs=h2sb[j // 2][:, BH * (j % 2):BH * (j % 2 + 1)],
                    start=True, stop=True,
                    tile_position=(0, 32 * j),
                )
            for j in range(4):
                nc.tensor.matmul(
                    out=shsl[32 * j:32 * (j + 1), :],
                    lhsT=w3s[:, 4 * g + j, 0:32],
                    rhs=h2sb[j // 2][:, BH * (j % 2):BH * (j % 2 + 1)],
                    start=True, stop=True,
                    tile_position=(0, 32 * j),
                )

            # reduce of the step TWO back: q/logs_sb finished ~a step
            # ago, so the reduce matmuls never stall the in-order PE
            # queue (a 1-step lag measurably head-of-line blocked the
            # next step's L1/L2 matmuls behind the GPSIMD q chain).
            if len(prevs) == 2:
                emit_reduce(prevs.pop(0))

            # E2 = exp(-2*logs)/2  (ACT)
            et = es.tile([128, BH], dt.bfloat16, tag="et")
            nc.scalar.activation(et[:], lgsl, AF.Exp,
                                 bias=ebias[:], scale=-2.0)
            # logs -> SBUF bf16 for the reduce (frees the L3 pair this
            # step; engine alternates for balance)
            lgev = es.tile([128, BH], dt.bfloat16, tag="lgev")
            if LGEV_ACT:
                nc.scalar.activation(lgev[:], lgsl, AF.Identity)
            else:
                nc.vector.tensor_copy(lgev[:], lgsl)
            # d = xg - shift  (DVE, PSUM operand)
            dtl = es.tile([128, BH], dt.bfloat16, tag="dt")
            nc.vector.tensor_sub(dtl[:], xgbs, shsl)
            # dd = d^2 runs in parallel with exp; q = dd * E2 = 0.5 u^2
            # (GPSIMD, SBUF-only) - only ONE GPSIMD op after exp, so the
            # q chain latency is half of the former u=d*E', q=u'^2.
            ddt = es.tile([128, BH], dt.bfloat16, tag="ddt")
            nc.gpsimd.tensor_mul(ddt[:], dtl[:], dtl[:])
            qt = es.tile([128, BH], dt.bfloat16, tag="qt")
            nc.gpsimd.tensor_mul(qt[:], ddt[:], et[:])

            prevs.append((qt, lgev, step))

        for p in prevs:
            emit_reduce(p)

    nc.compile()
    return nc


def _host_prep(inputs, W1, W2, Wout, idx, valid, M1, M2, Mout):
    import ml_dtypes

    bf16 = ml_dtypes.bfloat16
    f32 = np.float32

    idx = np.asarray(idx)
    valid = np.asarray(valid)
    vf = valid.astype(f32)                                  # [R, RMAX]
    Wm1 = (np.asarray(W1) * np.asarray(M1)).astype(f32)     # [R, 32, 128]
    Wm2 = (np.asarray(W2) * np.asarray(M2)).astype(f32)     # [R, 128, 128]
    Wm3 = (np.asarray(Wout) * np.asarray(Mout)).astype(f32)  # [R, 128, 64]
    Wsh = Wm3[:, :, 0::2]                                   # [R, 128, 32]
    Wlg = Wm3[:, :, 1::2]                                   # [R, 128, 32]

    w1 = np.zeros((128, NG, 128), f32)
    for g in range(NG):
        for j in range(4):
            w1[32 * j:32 * (j + 1), g, :] = Wm1[4 * g + j]
    w1 = w1.astype(bf16)
    w2 = np.ascontiguousarray(Wm2.transpose(1, 0, 2)).astype(bf16)  # [128,R,128]
    w3 = np.concatenate([Wsh, Wlg], axis=2)                 # [R, 128, 64]
    w3 = np.ascontiguousarray(w3.transpose(1, 0, 2)).astype(bf16)   # [128,R,64]

    negv = np.zeros((128, NG, 32), f32)
    for g in range(NG):
        for j in range(4):
            r = 4 * g + j
            negv[32 * j:32 * (j + 1), g, j] = -vf[r]
    negv = negv.astype(bf16)

    # cb[32*j + i, c] = -0.5*ln(2pi)*sum(v_r) for region r = 4g+i of
    # step s = 4c+j (g = 2c + j//2); the batched ll copy-out adds it as
    # a per-partition scalar.
    cbv = np.zeros((128, 4), f32)
    for c in range(4):
        for j in range(4):
            gg = 2 * c + j // 2
            for i in range(4):
                cbv[32 * j + i, c] = -0.5 * LN2PI * float(vf[4 * gg + i].sum())

    # host-side ragged gather: partition p of group g holds
    # x[:, idx[4g + p//32, p%32]] * valid, transposed to [feat, batch]
    rows = idx.reshape(NG, 4 * RMAX)                        # [NG, 128]
    vflat = vf.reshape(NG, 4 * RMAX)                        # [NG, 128]
    xT = np.asarray(inputs, dtype=f32).T                    # [D, B]
    xg_full = xT[rows.reshape(-1)] * vflat.reshape(-1, 1)   # [NG*128, B]
    xg_full = xg_full.reshape(NG, 128, B).astype(bf16)

    per_core = []
    for c in range(NCORES):
        sl = xg_full[:, :, c * BC:(c + 1) * BC]             # [NG, 128, BC]
        xg = np.ascontiguousarray(sl.transpose(1, 0, 2)).reshape(128, NG * BC)
        per_core.append({
            "xg": xg,
            "w1": w1, "w2": w2, "w3": w3,
            "negv": negv, "cb": cbv,
        })
    return per_core


def _get_compiled(idx, valid):
    key = (np.asarray(idx).tobytes(), np.asarray(valid).tobytes())
    if _cache.get("key") != key:
        _cache["key"] = key
        _cache["nc"] = _build_program(np.asarray(idx), np.asarray(valid))
    return _cache["nc"]


def _assemble(results):
    full = np.zeros((B, R), np.float32)
    for c in range(NCORES):
        o = results[c]["out"]                       # [4, NG*BC]
        o = o.reshape(4, NG, BC).transpose(2, 1, 0).reshape(BC, R)
        full[c * BC:(c + 1) * BC] = o
    return full[..., None]


def kernel(inputs, W1, W2, Wout, idx, valid, M1, M2, Mout):
    from concourse import bass_utils

    nc = _get_compiled(idx, valid)
    in_maps = _host_prep(inputs, W1, W2, Wout, idx, valid, M1, M2, Mout)
    res = bass_utils.run_bass_kernel_spmd(nc, in_maps, core_ids=list(range(NCORES)))
    out = _assemble(res.results)
    _cache["last_exec_time_ns"] = res.exec_time_ns
    return out


def kernel_profiled(inputs, W1, W2, Wout, idx, valid, M1, M2, Mout, tmpdir=None):
    """Like kernel() but requests an NTFF trace; returns (out, exec_time_ns)."""
    from concourse import bass_utils

    nc = _get_compiled(idx, valid)
    in_maps = _host_prep(inputs, W1, W2, Wout, idx, valid, M1, M2, Mout)
    res = bass_utils.run_bass_kernel_spmd(
        nc, in_maps, core_ids=list(range(NCORES)), trace=True, tmpdir=tmpdir,
    )
    out = _assemble(res.results)
    return out, res.exec_time_ns


# revision 25
# speedup vs baseline: 1.2353x; 1.0627x over previous
"""Trainium2 Bass kernel for nn_AutoregressiveFlowLayer (v22).

Computes, for batch x [B, D] and R ragged regions (padded to RMAX):
    xg   = x[:, idx] * valid                       [B, R, RMAX]
    h1   = relu(xg @ (W1*M1))                      [B, R, 128]
    h2   = relu(h1 @ (W2*M2))                      [B, R, 128]
    out  = h2 @ (Wout*Mout) -> (shift, log_s)      [B, R, RMAX, 2]
    u    = (xg - shift) * exp(-log_s)
    ll   = sum(valid * (-0.5 u^2 - 0.5 log(2pi) - log_s), -1)   [B, R, 1]

Sharding: data-parallel over batch across 8 NeuronCores; weights replicated.
idx/valid are baked into the compiled program (recompiled if they change).

v22 history:
  v20 (102.6us) was elementwise-bound: ACT 66.6us + DVE 66.1us of ~690ns
  PSUM-evacuation ops; PE 42us HAM-throttled from the resulting stalls.
  v21 (122.6us) halved the evacuation op count with 2-bank pair tiles but
  its 2-pair php ring serialized PE<->evac into lock-step (~43% engine
  utilization).  v22 keeps the pair savings and restores decoupling:
  - php = 3 pair bufs [128,1024]; per step 5 pair allocs: L1A, L1B,
    L2A, L2B, and L3 (logs half | shift half) - shift/logs fold into
    the same ring instead of owning dedicated banks.
  - p = q + logs is gone; instead logs is evacuated to SBUF (single-src
    op, schedulable on either engine) and the reduce becomes TWO
    accumulating matmuls ll = -(v.q) - (v.logs).  This frees the L3
    pair within its own step, breaking the cross-step dependency cycle
    through the q chain.
  - reduce matmuls of 4 consecutive steps write one pll bank at
    partition offsets 0/32/64/96 (tile_position=(0,32j), M=32 with
    zero-padded negv columns so the bank is fully initialized); one
    Identity+bias copy-out on ACT + 4 small DMAs per 4 steps.
  - PSUM banks: 3 pairs (6) + pll 2 = 8.
"""

import sys

import numpy as np

_TRN_REPO = "/opt/trn_rl_repo"
if _TRN_REPO not in sys.path:
    sys.path.insert(0, _TRN_REPO)

D = 1024
R = 32
RMAX = 32
H1 = 128
H2 = 128
B = 8192
NCORES = 8
BC = B // NCORES          # batch per core
NG = R // 4               # 8 groups of 4 regions
BH = 512                  # batch half-tile (one PSUM bank of fp32)
LN2PI = float(np.log(2.0 * np.pi))
EXP_BIAS = float(-np.log(2.0))  # exp(-2*logs + b) = exp(-2*logs)/2

_cache = {}


def _build_program(idx, valid):
    import concourse.mybir as mybir
    import concourse.tile as tile
    from concourse import bacc

    dt = mybir.dt
    AF = mybir.ActivationFunctionType

    nc = bacc.Bacc("TRN2", target_bir_lowering=False, debug=False)

    # ---- DRAM tensors (per-core inputs) ----
    xg_d = nc.dram_tensor("xg", [128, NG * BC], dt.bfloat16, kind="ExternalInput").ap()
    w1 = nc.dram_tensor("w1", [128, NG, 128], dt.bfloat16, kind="ExternalInput").ap()
    w2 = nc.dram_tensor("w2", [128, R, 128], dt.bfloat16, kind="ExternalInput").ap()
    w3 = nc.dram_tensor("w3", [128, R, 64], dt.bfloat16, kind="ExternalInput").ap()
    negv = nc.dram_tensor("negv", [128, NG, 32], dt.bfloat16, kind="ExternalInput").ap()
    wvp = nc.dram_tensor("wvp", [128, R, 32], dt.bfloat16, kind="ExternalInput").ap()
    cb = nc.dram_tensor("cb", [128, 4], dt.float32, kind="ExternalInput").ap()
    out_d = nc.dram_tensor("out", [4, NG * BC], dt.float32, kind="ExternalOutput").ap()

    from contextlib import ExitStack

    with tile.TileContext(nc) as tc, ExitStack() as ctx:
        singles = ctx.enter_context(tc.tile_pool(name="singles", bufs=1))
        h1pool = ctx.enter_context(tc.tile_pool(name="h1pool", bufs=4))
        # h2 pairs are also read by the 3-step-deferred reduce matmuls
        h2pool = ctx.enter_context(tc.tile_pool(name="h2pool", bufs=9))
        es = ctx.enter_context(tc.tile_pool(name="es", bufs=16))
        # PSUM: php = 3 pair slabs [128,1024] (2 banks each) cycling
        # L1A,L1B,L2A,L2B,L3 each step; pll = 2 banks, each collecting 4
        # steps' [4,512] ll rows at partition offsets 0/32/64/96.
        php = ctx.enter_context(tc.tile_pool(name="php", bufs=3, space="PSUM"))
        pll = ctx.enter_context(tc.tile_pool(name="pll", bufs=2, space="PSUM"))

        # ---- load constants into SBUF ----
        w1s = singles.tile([128, NG, 128], dt.bfloat16)
        w2s = singles.tile([128, R, 128], dt.bfloat16)
        w3s = singles.tile([128, R, 64], dt.bfloat16)
        negvs = singles.tile([128, NG, 32], dt.bfloat16)
        wvps = singles.tile([128, R, 32], dt.bfloat16)
        cbs = singles.tile([128, 4], dt.float32)

        # gathered ragged inputs (bf16, host-side gather): one tile per
        # group so compute on group g only waits for its own slab.
        xgb = []
        for g in range(NG):
            t = singles.tile([128, 1, BC], dt.bfloat16, tag=f"xgb{g}")
            xgb.append(t)

        # startup-critical slices first: step (0,0) needs only the first
        # batch half of group 0 and group 0's weights (~300KB), not the
        # full 3.75MB input set -> the first matmul starts ~3us earlier.
        nc.sync.dma_start(out=xgb[0][:, :, 0:BH], in_=xg_d[:, 0:BH])
        nc.sync.dma_start(out=w1s[:, 0, :], in_=w1[:, 0, :])
        nc.sync.dma_start(out=w2s[:, 0:4, :], in_=w2[:, 0:4, :])
        nc.sync.dma_start(out=w3s[:, 0:4, :], in_=w3[:, 0:4, :])
        nc.sync.dma_start(out=xgb[0][:, :, BH:BC], in_=xg_d[:, BH:BC])
        nc.sync.dma_start(out=negvs[:], in_=negv)
        nc.sync.dma_start(out=wvps[:], in_=wvp)
        nc.sync.dma_start(out=cbs[:], in_=cb)
        nc.sync.dma_start(out=xgb[1][:], in_=xg_d[:, BC:2 * BC])
        nc.sync.dma_start(out=w1s[:, 1:NG, :], in_=w1[:, 1:NG, :])
        nc.sync.dma_start(out=w2s[:, 4:R, :], in_=w2[:, 4:R, :])
        nc.sync.dma_start(out=w3s[:, 4:R, :], in_=w3[:, 4:R, :])
        for g in range(2, NG):
            nc.sync.dma_start(out=xgb[g][:], in_=xg_d[:, g * BC:(g + 1) * BC])

        # per-partition constant bias for the exp
        ebias = singles.tile([128, 1], dt.float32)
        nc.vector.memset(ebias[:], EXP_BIAS)

        # warm-load dummies: pull ACT_TABLE_LOAD + Q7 ucode load into the
        # preamble dead time.
        wl0 = singles.tile([1, 1], dt.bfloat16)
        nc.scalar.activation(wl0[:], ebias[0:1, 0:1], AF.Exp)
        wl1 = singles.tile([1, 1], dt.bfloat16)
        nc.gpsimd.tensor_mul(wl1[:], ebias[0:1, 0:1], ebias[0:1, 0:1])

        nh = BC // BH  # halves per core
        nsteps = NG * nh

        def relu(on_act, dst, src):
            if on_act:
                nc.scalar.activation(dst, src, AF.Relu)
            else:
                nc.vector.tensor_scalar_max(dst, src, 0.0)

        # deferred reduce of step `prev`: accumulating matmuls
        # ll4 = -(v.q) - sum_k v*logs into the shared pll bank at
        # partition offset 32*(s%4) (M=32, unused weight cols are zero
        # so the whole bank stays initialized).  The logs part uses the
        # host-precomputed wv_r = (Wlg_r @ v_r): sum_k v*logs = h2.wv,
        # one matmul per region - no logs evacuation op needed at all.
        # Every 4 steps: one ACT Identity+bias copy-out + 4 small DMAs.
        state = {"ll": None}

        def emit_reduce(prev):
            qt, h2pair, s = prev
            g = s // nh
            j = s % 4
            if j == 0:
                state["ll"] = pll.tile([128, BH], dt.float32, tag="ll",
                                       name="llt")
            llp = state["ll"][32 * j:32 * (j + 1), 0:BH]
            nc.tensor.matmul(
                out=llp, lhsT=negvs[:, g, :], rhs=qt[:],
                start=True, stop=False, tile_position=(0, 32 * j),
            )
            for i in range(4):
                nc.tensor.matmul(
                    out=llp, lhsT=wvps[:, 4 * g + i, :],
                    rhs=h2pair[i // 2][:, BH * (i % 2):BH * (i % 2 + 1)],
                    start=False, stop=(i == 3), tile_position=(0, 32 * j),
                )
            if j == 3:
                c = s // 4
                lls = singles.tile([128, BH], dt.float32, tag=f"lls{c}",
                                   name="lls")
                nc.scalar.activation(lls[:], state["ll"][:], AF.Identity,
                                     bias=cbs[:, c:c + 1])
                for jj in range(4):
                    nc.sync.dma_start(
                        out=out_d[:, 2 * c * BC + jj * BH:
                                  2 * c * BC + (jj + 1) * BH],
                        in_=lls[32 * jj:32 * jj + 4, :])

        prevs = []
        for step in range(nsteps):
            g, h = step // nh, step % nh
            b0 = h * BH
            xgbs = xgb[g][:, 0, b0:b0 + BH]

            # engine split: True = ACT.  DVE carries sub, ACT carries
            # exp (+ copy-out every 4th step).  Pair relus split 2/2.
            RELU_ACT = (True, False, True, False)

            # ---- L1: two pair slabs, 4 row-tiled K=32 matmuls
            l1p = [php.tile([128, 2 * BH], dt.float32, tag="ph", name="l1p")
                   for _ in range(2)]
            for j in range(4):
                nc.tensor.matmul(
                    out=l1p[j // 2][:, BH * (j % 2):BH * (j % 2 + 1)],
                    lhsT=w1s[32 * j:32 * (j + 1), g, :],
                    rhs=xgbs[32 * j:32 * (j + 1), :],
                    start=True, stop=True,
                    tile_position=(32 * j, 0),
                )
            h1sb = []
            for p in range(2):
                ht = h1pool.tile([128, 2 * BH], dt.bfloat16, tag="hsb",
                                 name="h1t")
                relu(RELU_ACT[p], ht[:], l1p[p][:])
                h1sb.append(ht)

            # ---- L2: two pair slabs, 4 dense K=128 matmuls
            l2p = [php.tile([128, 2 * BH], dt.float32, tag="ph", name="l2p")
                   for _ in range(2)]
            for j in range(4):
                nc.tensor.matmul(
                    out=l2p[j // 2][:, BH * (j % 2):BH * (j % 2 + 1)],
                    lhsT=w2s[:, 4 * g + j, :],
                    rhs=h1sb[j // 2][:, BH * (j % 2):BH * (j % 2 + 1)],
                    start=True, stop=True,
                    tile_position=(0, 0),
                )
            h2sb = []
            for p in range(2):
                ht = h2pool.tile([128, 2 * BH], dt.bfloat16, tag="hsb",
                                 name="h2t")
                relu(RELU_ACT[2 + p], ht[:], l2p[p][:])
                h2sb.append(ht)

            # ---- L3: ONE pair slab [logs | shift], col-tiled M=32
            # matmuls.  Logs half first so ACT's exp starts earlier.
            l3p = php.tile([128, 2 * BH], dt.float32, tag="ph", name="l3p")
            lgsl = l3p[:, 0:BH]
            shsl = l3p[:, BH:2 * BH]
            for j in range(4):
                nc.tensor.matmul(
                    out=lgsl[32 * j:32 * (j + 1), :],
                    lhsT=w3s[:, 4 * g + j, 32:64],
                    rhs=h2sb[j // 2][:, BH * (j % 2):BH * (j % 2 + 1)],
                    start=True, stop=True,
                    tile_position=(0, 32 * j),
                )
            for j in range(4):
                nc.tensor.matmul(
                    out=shsl[32 * j:32 * (j + 1), :],
                    lhsT=w3s[:, 4 * g + j, 0:32],
                    rhs=h2sb[j // 2][:, BH * (j % 2):BH * (j % 2 + 1)],
                    start=True, stop=True,
                    tile_position=(0, 32 * j),
                )

            # reduce of the step THREE back: its q is long finished, so
            # the reduce matmuls never stall the in-order PE queue (the
            # GPSIMD tail runs ~2 steps behind the PE since it is gated
            # by sub at the end of the DVE chain).
            if len(prevs) == 3:
                emit_reduce(prevs.pop(0))

            # E2 = exp(-2*logs)/2  (ACT)
            et = es.tile([128, BH], dt.bfloat16, tag="et")
            nc.scalar.activation(et[:], lgsl, AF.Exp,
                                 bias=ebias[:], scale=-2.0)
            # d = xg - shift  (DVE, PSUM operand)
            dtl = es.tile([128, BH], dt.bfloat16, tag="dt")
            nc.vector.tensor_sub(dtl[:], xgbs, shsl)
            # dd = d^2 runs in parallel with exp; q = dd * E2 = 0.5 u^2
            # (GPSIMD, SBUF-only) - only ONE GPSIMD op after exp, so the
            # q chain latency is half of the former u=d*E', q=u'^2.
            ddt = es.tile([128, BH], dt.bfloat16, tag="ddt")
            nc.gpsimd.tensor_mul(ddt[:], dtl[:], dtl[:])
            qt = es.tile([128, BH], dt.bfloat16, tag="qt")
            nc.gpsimd.tensor_mul(qt[:], ddt[:], et[:])

            prevs.append((qt, h2sb, step))

        for p in prevs:
            emit_reduce(p)

    nc.compile()
    return nc


def _host_prep(inputs, W1, W2, Wout, idx, valid, M1, M2, Mout):
    import ml_dtypes

    bf16 = ml_dtypes.bfloat16
    f32 = np.float32

    idx = np.asarray(idx)
    valid = np.asarray(valid)
    vf = valid.astype(f32)                                  # [R, RMAX]
    Wm1 = (np.asarray(W1) * np.asarray(M1)).astype(f32)     # [R, 32, 128]
    Wm2 = (np.asarray(W2) * np.asarray(M2)).astype(f32)     # [R, 128, 128]
    Wm3 = (np.asarray(Wout) * np.asarray(Mout)).astype(f32)  # [R, 128, 64]
    Wsh = Wm3[:, :, 0::2]                                   # [R, 128, 32]
    Wlg = Wm3[:, :, 1::2]                                   # [R, 128, 32]

    w1 = np.zeros((128, NG, 128), f32)
    for g in range(NG):
        for j in range(4):
            w1[32 * j:32 * (j + 1), g, :] = Wm1[4 * g + j]
    w1 = w1.astype(bf16)
    w2 = np.ascontiguousarray(Wm2.transpose(1, 0, 2)).astype(bf16)  # [128,R,128]
    w3 = np.concatenate([Wsh, Wlg], axis=2)                 # [R, 128, 64]
    w3 = np.ascontiguousarray(w3.transpose(1, 0, 2)).astype(bf16)   # [128,R,64]

    negv = np.zeros((128, NG, 32), f32)
    for g in range(NG):
        for j in range(4):
            r = 4 * g + j
            negv[32 * j:32 * (j + 1), g, j] = -vf[r]
    negv = negv.astype(bf16)

    # wvp[:, r, r%4] = -(Wlg_r @ v_r): the reduce matmul computes
    # -sum_k v*logs for region r as h2_r . wv_r (other cols zero).
    wvpv = np.zeros((128, R, 32), f32)
    for r in range(R):
        wvpv[:, r, r % 4] = -(Wlg[r] @ vf[r])
    wvpv = wvpv.astype(bf16)

    # cb[32*j + i, c] = -0.5*ln(2pi)*sum(v_r) for region r = 4g+i of
    # step s = 4c+j (g = 2c + j//2); the batched ll copy-out adds it as
    # a per-partition scalar.
    cbv = np.zeros((128, 4), f32)
    for c in range(4):
        for j in range(4):
            gg = 2 * c + j // 2
            for i in range(4):
                cbv[32 * j + i, c] = -0.5 * LN2PI * float(vf[4 * gg + i].sum())

    # host-side ragged gather: partition p of group g holds
    # x[:, idx[4g + p//32, p%32]] * valid, transposed to [feat, batch]
    rows = idx.reshape(NG, 4 * RMAX)                        # [NG, 128]
    vflat = vf.reshape(NG, 4 * RMAX)                        # [NG, 128]
    xT = np.asarray(inputs, dtype=f32).T                    # [D, B]
    xg_full = xT[rows.reshape(-1)] * vflat.reshape(-1, 1)   # [NG*128, B]
    xg_full = xg_full.reshape(NG, 128, B).astype(bf16)

    per_core = []
    for c in range(NCORES):
        sl = xg_full[:, :, c * BC:(c + 1) * BC]             # [NG, 128, BC]
        xg = np.ascontiguousarray(sl.transpose(1, 0, 2)).reshape(128, NG * BC)
        per_core.append({
            "xg": xg,
            "w1": w1, "w2": w2, "w3": w3,
            "negv": negv, "wvp": wvpv, "cb": cbv,
        })
    return per_core


def _get_compiled(idx, valid):
    key = (np.asarray(idx).tobytes(), np.asarray(valid).tobytes())
    if _cache.get("key") != key:
        _cache["key"] = key
        _cache["nc"] = _build_program(np.asarray(idx), np.asarray(valid))
    return _cache["nc"]


def _assemble(results):
    full = np.zeros((B, R), np.float32)
    for c in range(NCORES):
        o = results[c]["out"]                       # [4, NG*BC]
        o = o.reshape(4, NG, BC).transpose(2, 1, 0).reshape(BC, R)
        full[c * BC:(c + 1) * BC] = o
    return full[..., None]


def kernel(inputs, W1, W2, Wout, idx, valid, M1, M2, Mout):
    from concourse import bass_utils

    nc = _get_compiled(idx, valid)
    in_maps = _host_prep(inputs, W1, W2, Wout, idx, valid, M1, M2, Mout)
    res = bass_utils.run_bass_kernel_spmd(nc, in_maps, core_ids=list(range(NCORES)))
    out = _assemble(res.results)
    _cache["last_exec_time_ns"] = res.exec_time_ns
    return out


def kernel_profiled(inputs, W1, W2, Wout, idx, valid, M1, M2, Mout, tmpdir=None):
    """Like kernel() but requests an NTFF trace; returns (out, exec_time_ns)."""
    from concourse import bass_utils

    nc = _get_compiled(idx, valid)
    in_maps = _host_prep(inputs, W1, W2, Wout, idx, valid, M1, M2, Mout)
    res = bass_utils.run_bass_kernel_spmd(
        nc, in_maps, core_ids=list(range(NCORES)), trace=True, tmpdir=tmpdir,
    )
    out = _assemble(res.results)
    return out, res.exec_time_ns


# revision 29
# speedup vs baseline: 1.2982x; 1.0509x over previous
"""Trainium2 Bass kernel for nn_AutoregressiveFlowLayer (v22).

Computes, for batch x [B, D] and R ragged regions (padded to RMAX):
    xg   = x[:, idx] * valid                       [B, R, RMAX]
    h1   = relu(xg @ (W1*M1))                      [B, R, 128]
    h2   = relu(h1 @ (W2*M2))                      [B, R, 128]
    out  = h2 @ (Wout*Mout) -> (shift, log_s)      [B, R, RMAX, 2]
    u    = (xg - shift) * exp(-log_s)
    ll   = sum(valid * (-0.5 u^2 - 0.5 log(2pi) - log_s), -1)   [B, R, 1]

Sharding: data-parallel over batch across 8 NeuronCores; weights replicated.
idx/valid are baked into the compiled program (recompiled if they change).

v22 history:
  v20 (102.6us) was elementwise-bound: ACT 66.6us + DVE 66.1us of ~690ns
  PSUM-evacuation ops; PE 42us HAM-throttled from the resulting stalls.
  v21 (122.6us) halved the evacuation op count with 2-bank pair tiles but
  its 2-pair php ring serialized PE<->evac into lock-step (~43% engine
  utilization).  v22 keeps the pair savings and restores decoupling:
  - php = 3 pair bufs [128,1024]; per step 5 pair allocs: L1A, L1B,
    L2A, L2B, and L3 (logs half | shift half) - shift/logs fold into
    the same ring instead of owning dedicated banks.
  - p = q + logs is gone; instead logs is evacuated to SBUF (single-src
    op, schedulable on either engine) and the reduce becomes TWO
    accumulating matmuls ll = -(v.q) - (v.logs).  This frees the L3
    pair within its own step, breaking the cross-step dependency cycle
    through the q chain.
  - reduce matmuls of 4 consecutive steps write one pll bank at
    partition offsets 0/32/64/96 (tile_position=(0,32j), M=32 with
    zero-padded negv columns so the bank is fully initialized); one
    Identity+bias copy-out on ACT + 4 small DMAs per 4 steps.
  - PSUM banks: 3 pairs (6) + pll 2 = 8.
"""

import sys

import numpy as np

_TRN_REPO = "/opt/trn_rl_repo"
if _TRN_REPO not in sys.path:
    sys.path.insert(0, _TRN_REPO)

D = 1024
R = 32
RMAX = 32
H1 = 128
H2 = 128
B = 8192
NCORES = 8
BC = B // NCORES          # batch per core
NG = R // 4               # 8 groups of 4 regions
BH = 512                  # batch half-tile (one PSUM bank of fp32)
LN2PI = float(np.log(2.0 * np.pi))
EXP_BIAS = float(-np.log(2.0))  # exp(-2*logs + b) = exp(-2*logs)/2

_cache = {}


def _build_program(idx, valid):
    import concourse.mybir as mybir
    import concourse.tile as tile
    from concourse import bacc

    dt = mybir.dt
    AF = mybir.ActivationFunctionType

    nc = bacc.Bacc("TRN2", target_bir_lowering=False, debug=False)

    # ---- DRAM tensors (per-core inputs) ----
    xg_d = nc.dram_tensor("xg", [128, NG * BC], dt.bfloat16, kind="ExternalInput").ap()
    w1 = nc.dram_tensor("w1", [128, NG, 128], dt.bfloat16, kind="ExternalInput").ap()
    w2 = nc.dram_tensor("w2", [128, R, 128], dt.bfloat16, kind="ExternalInput").ap()
    w3 = nc.dram_tensor("w3", [128, R, 64], dt.bfloat16, kind="ExternalInput").ap()
    negv = nc.dram_tensor("negv", [128, NG, 32], dt.bfloat16, kind="ExternalInput").ap()
    wvp = nc.dram_tensor("wvp", [128, R, 32], dt.bfloat16, kind="ExternalInput").ap()
    cb = nc.dram_tensor("cb", [128, 4], dt.float32, kind="ExternalInput").ap()
    out_d = nc.dram_tensor("out", [4, NG * BC], dt.float32, kind="ExternalOutput").ap()

    from contextlib import ExitStack

    with tile.TileContext(nc) as tc, ExitStack() as ctx:
        singles = ctx.enter_context(tc.tile_pool(name="singles", bufs=1))
        h1pool = ctx.enter_context(tc.tile_pool(name="h1pool", bufs=4))
        # h2 pairs are also read by the 3-step-deferred reduce matmuls
        h2pool = ctx.enter_context(tc.tile_pool(name="h2pool", bufs=9))
        es = ctx.enter_context(tc.tile_pool(name="es", bufs=16))
        # PSUM: php = 2 pair slabs [128,1024] cycling L1A,L1B,L2A,L2B
        # (the 4-alloc/step rotation maps lane A always to buf0 and lane
        # B to buf1); pl3 = 1 pair [logs|shift] whose tail ops run one
        # step deferred; pll = 2 banks, each collecting 4 steps' [4,512]
        # ll rows at partition offsets 0/32/64/96.  4+2+2 = 8 banks.
        php = ctx.enter_context(tc.tile_pool(name="php", bufs=2, space="PSUM"))
        pl3 = ctx.enter_context(tc.tile_pool(name="pl3", bufs=1, space="PSUM"))
        pll = ctx.enter_context(tc.tile_pool(name="pll", bufs=2, space="PSUM"))

        # ---- load constants into SBUF ----
        w1s = singles.tile([128, NG, 128], dt.bfloat16)
        w2s = singles.tile([128, R, 128], dt.bfloat16)
        w3s = singles.tile([128, R, 64], dt.bfloat16)
        negvs = singles.tile([128, NG, 32], dt.bfloat16)
        wvps = singles.tile([128, R, 32], dt.bfloat16)
        cbs = singles.tile([128, 4], dt.float32)

        # gathered ragged inputs (bf16, host-side gather): one tile per
        # group so compute on group g only waits for its own slab.
        xgb = []
        for g in range(NG):
            t = singles.tile([128, 1, BC], dt.bfloat16, tag=f"xgb{g}")
            xgb.append(t)

        # startup-critical slices first: step (0,0) needs only the first
        # batch half of group 0 and group 0's weights (~300KB), not the
        # full 3.75MB input set -> the first matmul starts ~3us earlier.
        nc.sync.dma_start(out=xgb[0][:, :, 0:BH], in_=xg_d[:, 0:BH])
        nc.sync.dma_start(out=w1s[:, 0, :], in_=w1[:, 0, :])
        nc.sync.dma_start(out=w2s[:, 0:4, :], in_=w2[:, 0:4, :])
        nc.sync.dma_start(out=w3s[:, 0:4, :], in_=w3[:, 0:4, :])
        nc.sync.dma_start(out=xgb[0][:, :, BH:BC], in_=xg_d[:, BH:BC])
        nc.sync.dma_start(out=negvs[:], in_=negv)
        nc.sync.dma_start(out=wvps[:], in_=wvp)
        nc.sync.dma_start(out=cbs[:], in_=cb)
        nc.sync.dma_start(out=xgb[1][:], in_=xg_d[:, BC:2 * BC])
        nc.sync.dma_start(out=w1s[:, 1:NG, :], in_=w1[:, 1:NG, :])
        nc.sync.dma_start(out=w2s[:, 4:R, :], in_=w2[:, 4:R, :])
        nc.sync.dma_start(out=w3s[:, 4:R, :], in_=w3[:, 4:R, :])
        for g in range(2, NG):
            nc.sync.dma_start(out=xgb[g][:], in_=xg_d[:, g * BC:(g + 1) * BC])

        # per-partition constant bias for the exp
        ebias = singles.tile([128, 1], dt.float32)
        nc.vector.memset(ebias[:], EXP_BIAS)

        # warm-load dummies: pull ACT_TABLE_LOAD + Q7 ucode load into the
        # preamble dead time.
        wl0 = singles.tile([1, 1], dt.bfloat16)
        nc.scalar.activation(wl0[:], ebias[0:1, 0:1], AF.Exp)
        wl1 = singles.tile([1, 1], dt.bfloat16)
        nc.gpsimd.tensor_mul(wl1[:], ebias[0:1, 0:1], ebias[0:1, 0:1])

        nh = BC // BH  # halves per core
        nsteps = NG * nh

        def relu(on_act, dst, src):
            if on_act:
                nc.scalar.activation(dst, src, AF.Relu)
            else:
                nc.vector.tensor_scalar_max(dst, src, 0.0)

        # deferred reduce of step `prev`: accumulating matmuls
        # ll4 = -(v.q) - sum_k v*logs into the shared pll bank at
        # partition offset 32*(s%4) (M=32, unused weight cols are zero
        # so the whole bank stays initialized).  The logs part uses the
        # host-precomputed wv_r = (Wlg_r @ v_r): sum_k v*logs = h2.wv,
        # one matmul per region - no logs evacuation op needed at all.
        # Every 4 steps: one ACT Identity+bias copy-out + 4 small DMAs.
        state = {"ll": None}

        def emit_reduce(prev):
            qt, h2pair, s = prev
            g = s // nh
            j = s % 4
            if j == 0:
                state["ll"] = pll.tile([128, BH], dt.float32, tag="ll",
                                       name="llt")
            llp = state["ll"][32 * j:32 * (j + 1), 0:BH]
            nc.tensor.matmul(
                out=llp, lhsT=negvs[:, g, :], rhs=qt[:],
                start=True, stop=False, tile_position=(0, 32 * j),
            )
            for i in range(4):
                nc.tensor.matmul(
                    out=llp, lhsT=wvps[:, 4 * g + i, :],
                    rhs=h2pair[i // 2][:, BH * (i % 2):BH * (i % 2 + 1)],
                    start=False, stop=(i == 3), tile_position=(0, 32 * j),
                )
            if j == 3:
                c = s // 4
                lls = singles.tile([128, BH], dt.float32, tag=f"lls{c}",
                                   name="lls")
                nc.scalar.activation(lls[:], state["ll"][:], AF.Identity,
                                     bias=cbs[:, c:c + 1])
                for jj in range(4):
                    nc.sync.dma_start(
                        out=out_d[:, 2 * c * BC + jj * BH:
                                  2 * c * BC + (jj + 1) * BH],
                        in_=lls[32 * jj:32 * jj + 4, :])

        # deferred tail of step s: by the time it is emitted (one step
        # later) all its inputs are long computed, so the in-order ACT/
        # DVE queues never stall on it - the queue-order coupling
        # "next step's relus wait this step's exp/sub" disappears.
        def emit_tail(pend):
            l3p, xgbs, s = pend
            lgsl = l3p[:, 0:BH]
            shsl = l3p[:, BH:2 * BH]
            # E2 = exp(-2*logs)/2  (ACT)
            et = es.tile([128, BH], dt.bfloat16, tag="et", name="et")
            nc.scalar.activation(et[:], lgsl, AF.Exp,
                                 bias=ebias[:], scale=-2.0)
            # d = xg - shift  (DVE, PSUM operand)
            dtl = es.tile([128, BH], dt.bfloat16, tag="dt", name="dtl")
            nc.vector.tensor_sub(dtl[:], xgbs, shsl)
            # dd = d^2; q = dd * E2 = 0.5 u^2  (GPSIMD, SBUF-only)
            ddt = es.tile([128, BH], dt.bfloat16, tag="ddt", name="ddt")
            nc.gpsimd.tensor_mul(ddt[:], dtl[:], dtl[:])
            qt = es.tile([128, BH], dt.bfloat16, tag="qt", name="qt")
            nc.gpsimd.tensor_mul(qt[:], ddt[:], et[:])
            return qt

        # engine split: True = ACT.  DVE carries sub, ACT carries exp
        # (+ copy-out every 4th step).  Pair relus split 2/2.
        RELU_ACT = (True, False, True, False)

        def emit_L1(k):
            g, h = k // nh, k % nh
            xgbs = xgb[g][:, 0, h * BH:(h + 1) * BH]
            l1p = [php.tile([128, 2 * BH], dt.float32, tag="ph", name="l1p")
                   for _ in range(2)]
            for j in range(4):
                nc.tensor.matmul(
                    out=l1p[j // 2][:, BH * (j % 2):BH * (j % 2 + 1)],
                    lhsT=w1s[32 * j:32 * (j + 1), g, :],
                    rhs=xgbs[32 * j:32 * (j + 1), :],
                    start=True, stop=True,
                    tile_position=(32 * j, 0),
                )
            h1sb = []
            for p in range(2):
                ht = h1pool.tile([128, 2 * BH], dt.bfloat16, tag="hsb",
                                 name="h1t")
                relu(RELU_ACT[p], ht[:], l1p[p][:])
                h1sb.append(ht)
            return h1sb, xgbs

        def emit_L2(k, h1sb):
            g = k // nh
            l2p = [php.tile([128, 2 * BH], dt.float32, tag="ph", name="l2p")
                   for _ in range(2)]
            for j in range(4):
                nc.tensor.matmul(
                    out=l2p[j // 2][:, BH * (j % 2):BH * (j % 2 + 1)],
                    lhsT=w2s[:, 4 * g + j, :],
                    rhs=h1sb[j // 2][:, BH * (j % 2):BH * (j % 2 + 1)],
                    start=True, stop=True,
                    tile_position=(0, 0),
                )
            h2sb = []
            for p in range(2):
                ht = h2pool.tile([128, 2 * BH], dt.bfloat16, tag="hsb",
                                 name="h2t")
                relu(RELU_ACT[2 + p], ht[:], l2p[p][:])
                h2sb.append(ht)
            return h2sb

        def emit_L3(k, h2sb):
            # ONE pair slab [logs | shift], col-tiled M=32 matmuls.
            # Its only reader is the one-step-deferred tail, so these
            # matmuls are off the latency-critical path.
            g = k // nh
            l3p = pl3.tile([128, 2 * BH], dt.float32, tag="l3", name="l3p")
            for j in range(4):
                nc.tensor.matmul(
                    out=l3p[32 * j:32 * (j + 1), 0:BH],
                    lhsT=w3s[:, 4 * g + j, 32:64],
                    rhs=h2sb[j // 2][:, BH * (j % 2):BH * (j % 2 + 1)],
                    start=True, stop=True,
                    tile_position=(0, 32 * j),
                )
            for j in range(4):
                nc.tensor.matmul(
                    out=l3p[32 * j:32 * (j + 1), BH:2 * BH],
                    lhsT=w3s[:, 4 * g + j, 0:32],
                    rhs=h2sb[j // 2][:, BH * (j % 2):BH * (j % 2 + 1)],
                    start=True, stop=True,
                    tile_position=(0, 32 * j),
                )
            return l3p

        # Software-pipelined emission: per iteration k the engine queues
        # receive [L2(k)+relu2(k), tail(k-1), L1(k+1)+relu1(k+1), L3(k),
        # reduce(k-3)].  Every tail/reduce op's inputs are computed at
        # least a step earlier, so the in-order queues never couple one
        # step's latency chain into the next step's start.
        pend = None    # step whose tail is not yet emitted
        prevs = []     # steps whose reduce is not yet emitted
        h2keep = {}    # step -> h2 pair tiles (read by its reduce)
        h1_cur, xgbs_cur = emit_L1(0)
        for step in range(nsteps):
            h2sb = emit_L2(step, h1_cur)
            if pend is not None:
                qt = emit_tail(pend)
                prevs.append((qt, pend[2]))
            if step + 1 < nsteps:
                h1_next = emit_L1(step + 1)
            l3p = emit_L3(step, h2sb)
            if len(prevs) == 3:
                qt, s = prevs.pop(0)
                emit_reduce((qt, h2keep[s], s))
            h2keep[step] = h2sb
            pend = (l3p, xgbs_cur, step)
            if step + 1 < nsteps:
                h1_cur, xgbs_cur = h1_next

        qt = emit_tail(pend)
        prevs.append((qt, pend[2]))
        for qt, s in prevs:
            emit_reduce((qt, h2keep[s], s))

    nc.compile()
    return nc


def _host_prep(inputs, W1, W2, Wout, idx, valid, M1, M2, Mout):
    import ml_dtypes

    bf16 = ml_dtypes.bfloat16
    f32 = np.float32

    idx = np.asarray(idx)
    valid = np.asarray(valid)
    vf = valid.astype(f32)                                  # [R, RMAX]
    Wm1 = (np.asarray(W1) * np.asarray(M1)).astype(f32)     # [R, 32, 128]
    Wm2 = (np.asarray(W2) * np.asarray(M2)).astype(f32)     # [R, 128, 128]
    Wm3 = (np.asarray(Wout) * np.asarray(Mout)).astype(f32)  # [R, 128, 64]
    Wsh = Wm3[:, :, 0::2]                                   # [R, 128, 32]
    Wlg = Wm3[:, :, 1::2]                                   # [R, 128, 32]

    w1 = np.zeros((128, NG, 128), f32)
    for g in range(NG):
        for j in range(4):
            w1[32 * j:32 * (j + 1), g, :] = Wm1[4 * g + j]
    w1 = w1.astype(bf16)
    w2 = np.ascontiguousarray(Wm2.transpose(1, 0, 2)).astype(bf16)  # [128,R,128]
    w3 = np.concatenate([Wsh, Wlg], axis=2)                 # [R, 128, 64]
    w3 = np.ascontiguousarray(w3.transpose(1, 0, 2)).astype(bf16)   # [128,R,64]

    negv = np.zeros((128, NG, 32), f32)
    for g in range(NG):
        for j in range(4):
            r = 4 * g + j
            negv[32 * j:32 * (j + 1), g, j] = -vf[r]
    negv = negv.astype(bf16)

    # wvp[:, r, r%4] = -(Wlg_r @ v_r): the reduce matmul computes
    # -sum_k v*logs for region r as h2_r . wv_r (other cols zero).
    wvpv = np.zeros((128, R, 32), f32)
    for r in range(R):
        wvpv[:, r, r % 4] = -(Wlg[r] @ vf[r])
    wvpv = wvpv.astype(bf16)

    # cb[32*j + i, c] = -0.5*ln(2pi)*sum(v_r) for region r = 4g+i of
    # step s = 4c+j (g = 2c + j//2); the batched ll copy-out adds it as
    # a per-partition scalar.
    cbv = np.zeros((128, 4), f32)
    for c in range(4):
        for j in range(4):
            gg = 2 * c + j // 2
            for i in range(4):
                cbv[32 * j + i, c] = -0.5 * LN2PI * float(vf[4 * gg + i].sum())

    # host-side ragged gather: partition p of group g holds
    # x[:, idx[4g + p//32, p%32]] * valid, transposed to [feat, batch]
    rows = idx.reshape(NG, 4 * RMAX)                        # [NG, 128]
    vflat = vf.reshape(NG, 4 * RMAX)                        # [NG, 128]
    xT = np.asarray(inputs, dtype=f32).T                    # [D, B]
    xg_full = xT[rows.reshape(-1)] * vflat.reshape(-1, 1)   # [NG*128, B]
    xg_full = xg_full.reshape(NG, 128, B).astype(bf16)

    per_core = []
    for c in range(NCORES):
        sl = xg_full[:, :, c * BC:(c + 1) * BC]             # [NG, 128, BC]
        xg = np.ascontiguousarray(sl.transpose(1, 0, 2)).reshape(128, NG * BC)
        per_core.append({
            "xg": xg,
            "w1": w1, "w2": w2, "w3": w3,
            "negv": negv, "wvp": wvpv, "cb": cbv,
        })
    return per_core


def _get_compiled(idx, valid):
    key = (np.asarray(idx).tobytes(), np.asarray(valid).tobytes())
    if _cache.get("key") != key:
        _cache["key"] = key
        _cache["nc"] = _build_program(np.asarray(idx), np.asarray(valid))
    return _cache["nc"]


def _assemble(results):
    full = np.zeros((B, R), np.float32)
    for c in range(NCORES):
        o = results[c]["out"]                       # [4, NG*BC]
        o = o.reshape(4, NG, BC).transpose(2, 1, 0).reshape(BC, R)
        full[c * BC:(c + 1) * BC] = o
    return full[..., None]


def kernel(inputs, W1, W2, Wout, idx, valid, M1, M2, Mout):
    from concourse import bass_utils

    nc = _get_compiled(idx, valid)
    in_maps = _host_prep(inputs, W1, W2, Wout, idx, valid, M1, M2, Mout)
    res = bass_utils.run_bass_kernel_spmd(nc, in_maps, core_ids=list(range(NCORES)))
    out = _assemble(res.results)
    _cache["last_exec_time_ns"] = res.exec_time_ns
    return out


def kernel_profiled(inputs, W1, W2, Wout, idx, valid, M1, M2, Mout, tmpdir=None):
    """Like kernel() but requests an NTFF trace; returns (out, exec_time_ns)."""
    from concourse import bass_utils

    nc = _get_compiled(idx, valid)
    in_maps = _host_prep(inputs, W1, W2, Wout, idx, valid, M1, M2, Mout)
    res = bass_utils.run_bass_kernel_spmd(
        nc, in_maps, core_ids=list(range(NCORES)), trace=True, tmpdir=tmpdir,
    )
    out = _assemble(res.results)
    return out, res.exec_time_ns


# revision 32
# speedup vs baseline: 1.5568x; 1.1993x over previous
"""Trainium2 Bass kernel for nn_AutoregressiveFlowLayer (v22).

Computes, for batch x [B, D] and R ragged regions (padded to RMAX):
    xg   = x[:, idx] * valid                       [B, R, RMAX]
    h1   = relu(xg @ (W1*M1))                      [B, R, 128]
    h2   = relu(h1 @ (W2*M2))                      [B, R, 128]
    out  = h2 @ (Wout*Mout) -> (shift, log_s)      [B, R, RMAX, 2]
    u    = (xg - shift) * exp(-log_s)
    ll   = sum(valid * (-0.5 u^2 - 0.5 log(2pi) - log_s), -1)   [B, R, 1]

Sharding: data-parallel over batch across 8 NeuronCores; weights replicated.
idx/valid are baked into the compiled program (recompiled if they change).

v22 history:
  v20 (102.6us) was elementwise-bound: ACT 66.6us + DVE 66.1us of ~690ns
  PSUM-evacuation ops; PE 42us HAM-throttled from the resulting stalls.
  v21 (122.6us) halved the evacuation op count with 2-bank pair tiles but
  its 2-pair php ring serialized PE<->evac into lock-step (~43% engine
  utilization).  v22 keeps the pair savings and restores decoupling:
  - php = 3 pair bufs [128,1024]; per step 5 pair allocs: L1A, L1B,
    L2A, L2B, and L3 (logs half | shift half) - shift/logs fold into
    the same ring instead of owning dedicated banks.
  - p = q + logs is gone; instead logs is evacuated to SBUF (single-src
    op, schedulable on either engine) and the reduce becomes TWO
    accumulating matmuls ll = -(v.q) - (v.logs).  This frees the L3
    pair within its own step, breaking the cross-step dependency cycle
    through the q chain.
  - reduce matmuls of 4 consecutive steps write one pll bank at
    partition offsets 0/32/64/96 (tile_position=(0,32j), M=32 with
    zero-padded negv columns so the bank is fully initialized); one
    Identity+bias copy-out on ACT + 4 small DMAs per 4 steps.
  - PSUM banks: 3 pairs (6) + pll 2 = 8.
"""

import sys

import numpy as np

_TRN_REPO = "/opt/trn_rl_repo"
if _TRN_REPO not in sys.path:
    sys.path.insert(0, _TRN_REPO)

D = 1024
R = 32
RMAX = 32
H1 = 128
H2 = 128
B = 8192
NCORES = 8
BC = B // NCORES          # batch per core
NG = R // 4               # 8 groups of 4 regions
BH = 512                  # batch half-tile (one PSUM bank of fp32)
LN2PI = float(np.log(2.0 * np.pi))
EXP_BIAS = float(-np.log(2.0))  # exp(-2*logs + b) = exp(-2*logs)/2

_cache = {}


def _build_program(idx, valid):
    import concourse.mybir as mybir
    import concourse.tile as tile
    from concourse import bacc

    dt = mybir.dt
    AF = mybir.ActivationFunctionType

    nc = bacc.Bacc("TRN2", target_bir_lowering=False, debug=False)

    # ---- DRAM tensors (per-core inputs) ----
    xg_d = nc.dram_tensor("xg", [128, NG * BC], dt.bfloat16, kind="ExternalInput").ap()
    w1 = nc.dram_tensor("w1", [128, NG, 128], dt.bfloat16, kind="ExternalInput").ap()
    w2 = nc.dram_tensor("w2", [128, R, 128], dt.bfloat16, kind="ExternalInput").ap()
    w3 = nc.dram_tensor("w3", [128, R, 64], dt.bfloat16, kind="ExternalInput").ap()
    negv = nc.dram_tensor("negv", [128, NG, 32], dt.bfloat16, kind="ExternalInput").ap()
    wvp = nc.dram_tensor("wvp", [128, R, 32], dt.bfloat16, kind="ExternalInput").ap()
    cb = nc.dram_tensor("cb", [128, 4], dt.float32, kind="ExternalInput").ap()
    out_d = nc.dram_tensor("out", [4, NG * BC], dt.float32, kind="ExternalOutput").ap()

    from contextlib import ExitStack

    with tile.TileContext(nc) as tc, ExitStack() as ctx:
        singles = ctx.enter_context(tc.tile_pool(name="singles", bufs=1))
        h1pool = ctx.enter_context(tc.tile_pool(name="h1pool", bufs=4))
        # h2 pairs are also read by the 5-step-deferred reduce matmuls
        h2pool = ctx.enter_context(tc.tile_pool(name="h2pool", bufs=14))
        es = ctx.enter_context(tc.tile_pool(name="es", bufs=26))
        # PSUM: php = 2 pair slabs [128,1024] cycling L1A,L1B,L2A,L2B
        # (the 4-alloc/step rotation maps lane A always to buf0 and lane
        # B to buf1); pl3 = 1 pair [logs|shift] whose tail ops run one
        # step deferred; pll = 2 banks, each collecting 4 steps' [4,512]
        # ll rows at partition offsets 0/32/64/96.  4+2+2 = 8 banks.
        php = ctx.enter_context(tc.tile_pool(name="php", bufs=2, space="PSUM"))
        pl3 = ctx.enter_context(tc.tile_pool(name="pl3", bufs=1, space="PSUM"))
        pll = ctx.enter_context(tc.tile_pool(name="pll", bufs=2, space="PSUM"))

        # ---- load constants into SBUF ----
        w1s = singles.tile([128, NG, 128], dt.bfloat16)
        w2s = singles.tile([128, R, 128], dt.bfloat16)
        w3s = singles.tile([128, R, 64], dt.bfloat16)
        negvs = singles.tile([128, NG, 32], dt.bfloat16)
        wvps = singles.tile([128, R, 32], dt.bfloat16)
        cbs = singles.tile([128, 4], dt.float32)

        # gathered ragged inputs (bf16, host-side gather): one tile per
        # group so compute on group g only waits for its own slab.
        xgb = []
        for g in range(NG):
            t = singles.tile([128, 1, BC], dt.bfloat16, tag=f"xgb{g}")
            xgb.append(t)

        # startup-critical slices first: step (0,0) needs only the first
        # batch half of group 0 and group 0's weights (~300KB), not the
        # full 3.75MB input set -> the first matmul starts ~3us earlier.
        nc.sync.dma_start(out=xgb[0][:, :, 0:BH], in_=xg_d[:, 0:BH])
        nc.sync.dma_start(out=w1s[:, 0, :], in_=w1[:, 0, :])
        nc.sync.dma_start(out=w2s[:, 0:4, :], in_=w2[:, 0:4, :])
        nc.sync.dma_start(out=w3s[:, 0:4, :], in_=w3[:, 0:4, :])
        nc.sync.dma_start(out=xgb[0][:, :, BH:BC], in_=xg_d[:, BH:BC])
        nc.sync.dma_start(out=negvs[:], in_=negv)
        nc.sync.dma_start(out=wvps[:], in_=wvp)
        nc.sync.dma_start(out=cbs[:], in_=cb)
        nc.sync.dma_start(out=xgb[1][:], in_=xg_d[:, BC:2 * BC])
        nc.sync.dma_start(out=w1s[:, 1:NG, :], in_=w1[:, 1:NG, :])
        nc.sync.dma_start(out=w2s[:, 4:R, :], in_=w2[:, 4:R, :])
        nc.sync.dma_start(out=w3s[:, 4:R, :], in_=w3[:, 4:R, :])
        for g in range(2, NG):
            nc.sync.dma_start(out=xgb[g][:], in_=xg_d[:, g * BC:(g + 1) * BC])

        # per-partition constant bias for the exp
        ebias = singles.tile([128, 1], dt.float32)
        nc.vector.memset(ebias[:], EXP_BIAS)

        # warm-load dummies: pull ACT_TABLE_LOAD + Q7 ucode load into the
        # preamble dead time.
        wl0 = singles.tile([1, 1], dt.bfloat16)
        nc.scalar.activation(wl0[:], ebias[0:1, 0:1], AF.Exp)
        wl1 = singles.tile([1, 1], dt.bfloat16)
        nc.gpsimd.tensor_mul(wl1[:], ebias[0:1, 0:1], ebias[0:1, 0:1])

        nh = BC // BH  # halves per core
        nsteps = NG * nh

        def relu(on_act, dst, src):
            if on_act:
                nc.scalar.activation(dst, src, AF.Relu)
            else:
                nc.vector.tensor_scalar_max(dst, src, 0.0)

        # deferred reduce of step `prev`: accumulating matmuls
        # ll4 = -(v.q) - sum_k v*logs into the shared pll bank at
        # partition offset 32*(s%4) (M=32, unused weight cols are zero
        # so the whole bank stays initialized).  The logs part uses the
        # host-precomputed wv_r = (Wlg_r @ v_r): sum_k v*logs = h2.wv,
        # one matmul per region - no logs evacuation op needed at all.
        # Every 4 steps: one ACT Identity+bias copy-out + 4 small DMAs.
        state = {"ll": None}

        def emit_reduce(prev):
            qt, h2pair, s = prev
            g = s // nh
            j = s % 4
            if j == 0:
                state["ll"] = pll.tile([128, BH], dt.float32, tag="ll",
                                       name="llt")
            llp = state["ll"][32 * j:32 * (j + 1), 0:BH]
            nc.tensor.matmul(
                out=llp, lhsT=negvs[:, g, :], rhs=qt[:],
                start=True, stop=False, tile_position=(0, 32 * j),
            )
            for i in range(4):
                nc.tensor.matmul(
                    out=llp, lhsT=wvps[:, 4 * g + i, :],
                    rhs=h2pair[i // 2][:, BH * (i % 2):BH * (i % 2 + 1)],
                    start=False, stop=(i == 3), tile_position=(0, 32 * j),
                )
            if j == 3:
                c = s // 4
                lls = singles.tile([128, BH], dt.float32, tag=f"lls{c}",
                                   name="lls")
                nc.scalar.activation(lls[:], state["ll"][:], AF.Identity,
                                     bias=cbs[:, c:c + 1])
                for jj in range(4):
                    nc.sync.dma_start(
                        out=out_d[:, 2 * c * BC + jj * BH:
                                  2 * c * BC + (jj + 1) * BH],
                        in_=lls[32 * jj:32 * jj + 4, :])

        # deferred tail of step s: by the time it is emitted (one step
        # later) all its inputs are long computed, so the in-order ACT/
        # DVE queues never stall on it - the queue-order coupling
        # "next step's relus wait this step's exp/sub" disappears.
        def emit_tail(pend):
            l3p, xgbs, s = pend
            lgsl = l3p[:, 0:BH]
            shsl = l3p[:, BH:2 * BH]
            # E2 = exp(-2*logs)/2  (ACT)
            et = es.tile([128, BH], dt.bfloat16, tag="et", name="et")
            nc.scalar.activation(et[:], lgsl, AF.Exp,
                                 bias=ebias[:], scale=-2.0)
            # d = xg - shift  (DVE, PSUM operand)
            dtl = es.tile([128, BH], dt.bfloat16, tag="dt", name="dtl")
            nc.vector.tensor_sub(dtl[:], xgbs, shsl)
            # dd = d^2 on DVE right behind sub (bf16 SBUF tensor_tensor
            # runs in 2x mode, ~330ns); q = dd * E2 = 0.5 u^2 (GPSIMD)
            ddt = es.tile([128, BH], dt.bfloat16, tag="ddt", name="ddt")
            nc.vector.tensor_mul(ddt[:], dtl[:], dtl[:])
            qt = es.tile([128, BH], dt.bfloat16, tag="qt", name="qt")
            nc.gpsimd.tensor_mul(qt[:], ddt[:], et[:])
            return qt

        # engine split: True = ACT.  DVE carries sub, ACT carries exp
        # (+ copy-out every 4th step).  Pair relus split 2/2.
        RELU_ACT = (True, False, True, False)

        def emit_L1(k):
            g, h = k // nh, k % nh
            xgbs = xgb[g][:, 0, h * BH:(h + 1) * BH]
            l1p = [php.tile([128, 2 * BH], dt.float32, tag="ph", name="l1p")
                   for _ in range(2)]
            for j in range(4):
                nc.tensor.matmul(
                    out=l1p[j // 2][:, BH * (j % 2):BH * (j % 2 + 1)],
                    lhsT=w1s[32 * j:32 * (j + 1), g, :],
                    rhs=xgbs[32 * j:32 * (j + 1), :],
                    start=True, stop=True,
                    tile_position=(32 * j, 0),
                )
            h1sb = []
            for p in range(2):
                ht = h1pool.tile([128, 2 * BH], dt.bfloat16, tag="hsb",
                                 name="h1t")
                relu(RELU_ACT[p], ht[:], l1p[p][:])
                h1sb.append(ht)
            return h1sb, xgbs

        def emit_L2(k, h1sb):
            g = k // nh
            l2p = [php.tile([128, 2 * BH], dt.float32, tag="ph", name="l2p")
                   for _ in range(2)]
            for j in range(4):
                nc.tensor.matmul(
                    out=l2p[j // 2][:, BH * (j % 2):BH * (j % 2 + 1)],
                    lhsT=w2s[:, 4 * g + j, :],
                    rhs=h1sb[j // 2][:, BH * (j % 2):BH * (j % 2 + 1)],
                    start=True, stop=True,
                    tile_position=(0, 0),
                )
            h2sb = []
            for p in range(2):
                ht = h2pool.tile([128, 2 * BH], dt.bfloat16, tag="hsb",
                                 name="h2t")
                relu(RELU_ACT[2 + p], ht[:], l2p[p][:])
                h2sb.append(ht)
            return h2sb

        def emit_L3(k, h2sb):
            # ONE pair slab [logs | shift], col-tiled M=32 matmuls.
            # Its only reader is the one-step-deferred tail, so these
            # matmuls are off the latency-critical path.
            g = k // nh
            l3p = pl3.tile([128, 2 * BH], dt.float32, tag="l3", name="l3p")
            for j in range(4):
                nc.tensor.matmul(
                    out=l3p[32 * j:32 * (j + 1), 0:BH],
                    lhsT=w3s[:, 4 * g + j, 32:64],
                    rhs=h2sb[j // 2][:, BH * (j % 2):BH * (j % 2 + 1)],
                    start=True, stop=True,
                    tile_position=(0, 32 * j),
                )
            for j in range(4):
                nc.tensor.matmul(
                    out=l3p[32 * j:32 * (j + 1), BH:2 * BH],
                    lhsT=w3s[:, 4 * g + j, 0:32],
                    rhs=h2sb[j // 2][:, BH * (j % 2):BH * (j % 2 + 1)],
                    start=True, stop=True,
                    tile_position=(0, 32 * j),
                )
            return l3p

        # Software-pipelined emission: per iteration k the engine queues
        # receive [L2(k)+relu2(k), tail(k-1), L1(k+1)+relu1(k+1), L3(k),
        # reduce(k-3)].  Every tail/reduce op's inputs are computed at
        # least a step earlier, so the in-order queues never couple one
        # step's latency chain into the next step's start.
        pend = None    # step whose tail is not yet emitted
        prevs = []     # steps whose reduce is not yet emitted
        h2keep = {}    # step -> h2 pair tiles (read by its reduce)
        h1_cur, xgbs_cur = emit_L1(0)
        for step in range(nsteps):
            h2sb = emit_L2(step, h1_cur)
            if pend is not None:
                qt = emit_tail(pend)
                prevs.append((qt, pend[2]))
            if step + 1 < nsteps:
                h1_next = emit_L1(step + 1)
            l3p = emit_L3(step, h2sb)
            if len(prevs) == 5:
                qt, s = prevs.pop(0)
                emit_reduce((qt, h2keep[s], s))
            h2keep[step] = h2sb
            pend = (l3p, xgbs_cur, step)
            if step + 1 < nsteps:
                h1_cur, xgbs_cur = h1_next

        qt = emit_tail(pend)
        prevs.append((qt, pend[2]))
        for qt, s in prevs:
            emit_reduce((qt, h2keep[s], s))

    nc.compile()
    return nc


def _host_prep(inputs, W1, W2, Wout, idx, valid, M1, M2, Mout):
    import ml_dtypes

    bf16 = ml_dtypes.bfloat16
    f32 = np.float32

    idx = np.asarray(idx)
    valid = np.asarray(valid)
    vf = valid.astype(f32)                                  # [R, RMAX]
    Wm1 = (np.asarray(W1) * np.asarray(M1)).astype(f32)     # [R, 32, 128]
    Wm2 = (np.asarray(W2) * np.asarray(M2)).astype(f32)     # [R, 128, 128]
    Wm3 = (np.asarray(Wout) * np.asarray(Mout)).astype(f32)  # [R, 128, 64]
    Wsh = Wm3[:, :, 0::2]                                   # [R, 128, 32]
    Wlg = Wm3[:, :, 1::2]                                   # [R, 128, 32]

    w1 = np.zeros((128, NG, 128), f32)
    for g in range(NG):
        for j in range(4):
            w1[32 * j:32 * (j + 1), g, :] = Wm1[4 * g + j]
    w1 = w1.astype(bf16)
    w2 = np.ascontiguousarray(Wm2.transpose(1, 0, 2)).astype(bf16)  # [128,R,128]
    w3 = np.concatenate([Wsh, Wlg], axis=2)                 # [R, 128, 64]
    w3 = np.ascontiguousarray(w3.transpose(1, 0, 2)).astype(bf16)   # [128,R,64]

    negv = np.zeros((128, NG, 32), f32)
    for g in range(NG):
        for j in range(4):
            r = 4 * g + j
            negv[32 * j:32 * (j + 1), g, j] = -vf[r]
    negv = negv.astype(bf16)

    # wvp[:, r, r%4] = -(Wlg_r @ v_r): the reduce matmul computes
    # -sum_k v*logs for region r as h2_r . wv_r (other cols zero).
    wvpv = np.zeros((128, R, 32), f32)
    for r in range(R):
        wvpv[:, r, r % 4] = -(Wlg[r] @ vf[r])
    wvpv = wvpv.astype(bf16)

    # cb[32*j + i, c] = -0.5*ln(2pi)*sum(v_r) for region r = 4g+i of
    # step s = 4c+j (g = 2c + j//2); the batched ll copy-out adds it as
    # a per-partition scalar.
    cbv = np.zeros((128, 4), f32)
    for c in range(4):
        for j in range(4):
            gg = 2 * c + j // 2
            for i in range(4):
                cbv[32 * j + i, c] = -0.5 * LN2PI * float(vf[4 * gg + i].sum())

    # host-side ragged gather: partition p of group g holds
    # x[:, idx[4g + p//32, p%32]] * valid, transposed to [feat, batch]
    rows = idx.reshape(NG, 4 * RMAX)                        # [NG, 128]
    vflat = vf.reshape(NG, 4 * RMAX)                        # [NG, 128]
    xT = np.asarray(inputs, dtype=f32).T                    # [D, B]
    xg_full = xT[rows.reshape(-1)] * vflat.reshape(-1, 1)   # [NG*128, B]
    xg_full = xg_full.reshape(NG, 128, B).astype(bf16)

    per_core = []
    for c in range(NCORES):
        sl = xg_full[:, :, c * BC:(c + 1) * BC]             # [NG, 128, BC]
        xg = np.ascontiguousarray(sl.transpose(1, 0, 2)).reshape(128, NG * BC)
        per_core.append({
            "xg": xg,
            "w1": w1, "w2": w2, "w3": w3,
            "negv": negv, "wvp": wvpv, "cb": cbv,
        })
    return per_core


def _get_compiled(idx, valid):
    key = (np.asarray(idx).tobytes(), np.asarray(valid).tobytes())
    if _cache.get("key") != key:
        _cache["key"] = key
        _cache["nc"] = _build_program(np.asarray(idx), np.asarray(valid))
    return _cache["nc"]


def _assemble(results):
    full = np.zeros((B, R), np.float32)
    for c in range(NCORES):
        o = results[c]["out"]                       # [4, NG*BC]
        o = o.reshape(4, NG, BC).transpose(2, 1, 0).reshape(BC, R)
        full[c * BC:(c + 1) * BC] = o
    return full[..., None]


def kernel(inputs, W1, W2, Wout, idx, valid, M1, M2, Mout):
    from concourse import bass_utils

    nc = _get_compiled(idx, valid)
    in_maps = _host_prep(inputs, W1, W2, Wout, idx, valid, M1, M2, Mout)
    res = bass_utils.run_bass_kernel_spmd(nc, in_maps, core_ids=list(range(NCORES)))
    out = _assemble(res.results)
    _cache["last_exec_time_ns"] = res.exec_time_ns
    return out


def kernel_profiled(inputs, W1, W2, Wout, idx, valid, M1, M2, Mout, tmpdir=None):
    """Like kernel() but requests an NTFF trace; returns (out, exec_time_ns)."""
    from concourse import bass_utils

    nc = _get_compiled(idx, valid)
    in_maps = _host_prep(inputs, W1, W2, Wout, idx, valid, M1, M2, Mout)
    res = bass_utils.run_bass_kernel_spmd(
        nc, in_maps, core_ids=list(range(NCORES)), trace=True, tmpdir=tmpdir,
    )
    out = _assemble(res.results)
    return out, res.exec_time_ns
